# revision 94
# baseline (speedup 1.0000x reference)
"""Trainium2 Bass kernel for nn_Block_self_attention_inter_intra_3D.

Math: the reference loops 36 overlapping windows (i,j in 0..2, z in 0..3) of a
(2,64,48,48,16) volume, runs channel-projected position attention inside each
window (reading the ORIGINAL x), and writes results back last-write-wins.
Each window "owns" exactly its local [0:16,0:16,0:4] sub-box (1024 query
positions); keys are all N window positions (N in {3456,2304,1536,1024}).

Sharding: 72 (window,batch) tasks -> 8 cores x 9 uniform slots
(3x N=3456, 4x N=2304, 2x N=1536; the two N=1024 tasks are padded to 1536
with a -80 additive key mask folded in as an extra contraction channel).

Per-task device pipeline, processed in two 512-query halves.  Both q- and
k-projections are fused into ONE projection of the queries, and the
v-projection is factored through the raw inputs (G-trick):
  qk_aug = A^T xq' + c'     A = Wk^T Wq; xq' = xq + bv (host); c' absorbs
                            bq/bk/bv so the residual add also applies bv
  energyT[m,n] = xk66[:,m]^T qk_aug[:,n]      (f32r, 512-row matmuls)
  ex = exp(energyT)         split across TWO engines: ACT runs exact exp
                            (bf16 out); DVE runs Schraudolph exp -- one
                            tensor_scalar producing int16 bf16-bit-patterns
  G[q, c]  (query-partition layout, 4x 128-query chunks x 65 cols/bank):
      G[:, qc*65+c] += ex_chunk^T @ xkt   (bf16, 65 rows per matmul -- this
                            orientation costs 65 PE cycles/chunk instead of
                            512, cutting att@v PE time by ~8x)
      col 64 of each chunk = sum(ex) via a ones-column in xkt
  normalize per partition:  gT = G * (1/sumexp)[q]  (scale sits on the
                            PARTITION axis, so it is a cheap scalar-AP op)
  transpose gT (PE, bf16, 4x 128x64 blocks) -> T[c, q]
  out = Wv^T T + xq'        (bf16 Wv matmul + DVE add, residual+bv via xq')

PSUM (8 banks): 3 x (128,1024) energy ring tiles = 6 banks + 1 bank G
accumulator + 1 bank shared transient slot (qk halves / transpose target /
Wv out).  Hardware constraints discovered on the way (each verified with a
micro-kernel on the real device):
  - start=True zeroes the ENTIRE psum bank -> exactly one accumulation
    chain per bank, one start per half, explicit scheduler edges so the
    starting matmul runs before the other t=0 chunk matmuls;
  - the dep tracker treats start=True matmuls as whole-tile accesses, so
    ring slots must be separate pool tiles, not offsets into one tile;
  - GPSIMD cannot touch PSUM at all, so every PSUM-side element-wise op
    lives on ACT or DVE: normalize/recip/residual-add on DVE, T-evac and
    qk bias-add (Identity + bias AP) on ACT, balanced against the exp
    split (DVE 76 of 174 full groups, Bresenham-interleaved so the ring
    always has both consumers draining in parallel).
The last task's final half splits its tail into two query-pair chains
(the second borrowing the then-free G bank) to shorten the end-of-kernel
serial chain.  Sim/HW: 205.7us (prev session baseline) -> 155.6us.
"""

import os
import sys

sys.path.insert(0, "/opt/trn_rl_repo")

from contextlib import ExitStack

import ml_dtypes
import numpy as np

import concourse.bacc as bacc
import concourse.mybir as mybir
import concourse.tile as tile
from concourse.bass_utils import run_bass_kernel_spmd
from concourse.tile_rust import add_dep_helper

F32 = mybir.dt.float32
F32R = mybir.dt.float32r
I16 = mybir.dt.int16
BF16 = mybir.dt.bfloat16

N_CORES = 8
NQ = 1024
HQ = 512
SLOT_NK = [3456, 3456, 3456, 2304, 2304, 2304, 2304, 1536, 1536]
# mask energy: exp(-80) ~ 1.8e-35 vanishes in both the exact exp and the
# Schraudolph path (whose int16 arithmetic would overflow around -1e3)
MASK_NEG = -80.0

# Schraudolph exp on DVE: exp(x) ~ bitcast_bf16(int16(x * A + B)), computed
# in f32 with a round-to-nearest int16 store (HW-verified).  B folds a -7.29
# shift that centers the (1+f)/2^f interpolation error (3.45% RMS).
SCH_A = 128.0 / float(np.log(2.0))          # 2^7 / ln 2
SCH_B = float(127 * (1 << 7)) - 477500.0 / (1 << 16)
# of the 174 full-width exp groups DVE takes 76 (see module docstring)
DVE_SHARE = 66.0 / 174.0

B, C, H, W, T = 2, 64, 48, 48, 16


def _win(i):
    s = 16 * i
    return s, min(s + 24, 48) - s


def _win_z(z):
    s = 4 * z
    return s, min(s + 6, 16) - s


def _task_lists():
    t3456 = [(b, i, j, z) for b in (0, 1) for i in (0, 1) for j in (0, 1)
             for z in (0, 1, 2)]
    t2304 = ([(b, i, j, 3) for b in (0, 1) for i in (0, 1) for j in (0, 1)] +
             [(b, i, 2, z) for b in (0, 1) for i in (0, 1) for z in (0, 1, 2)] +
             [(b, 2, j, z) for b in (0, 1) for j in (0, 1) for z in (0, 1, 2)])
    t1536 = ([(b, i, 2, 3) for b in (0, 1) for i in (0, 1)] +
             [(b, 2, j, 3) for b in (0, 1) for j in (0, 1)] +
             [(b, 2, 2, z) for b in (0, 1) for z in (0, 1, 2)])
    t1024 = [(b, 2, 2, 3) for b in (0, 1)]
    assert len(t3456) == 24 and len(t2304) == 32
    assert len(t1536) == 14 and len(t1024) == 2
    tail = t1536 + t1024
    per_core = []
    for c in range(N_CORES):
        per_core.append(t3456[3 * c:3 * c + 3] + t2304[4 * c:4 * c + 4] +
                        tail[2 * c:2 * c + 2])
    return per_core


TASKS = _task_lists()


def _emit(nc, tc, ctx, aps, reps):
    wp = ctx.enter_context(tc.tile_pool(name="wp", bufs=1))
    sb = ctx.enter_context(tc.tile_pool(name="sb", bufs=3))
    sbt = ctx.enter_context(tc.tile_pool(name="sbt", bufs=5))
    expp = ctx.enter_context(tc.tile_pool(name="expp", bufs=10))
    # PSUM: E ring 3 x (128,1024) tiles = 6 banks + G 1 bank + transient
    # slot 1 bank = 8.  Separate ring TILES (not offsets in one tile):
    # start=True matmuls act on the whole bank, and the dep tracker
    # accordingly treats them as whole-tile accesses -- one shared tile
    # would serialize the entire stream.
    psE = ctx.enter_context(tc.tile_pool(name="psE", bufs=3, space="PSUM"))
    psG = ctx.enter_context(tc.tile_pool(name="psG", bufs=1, space="PSUM"))
    psO = ctx.enter_context(tc.tile_pool(name="psO", bufs=1, space="PSUM"))

    Exp = mybir.ActivationFunctionType.Exp
    Copy = mybir.ActivationFunctionType.Copy
    Ident = mybir.ActivationFunctionType.Identity
    Mult = mybir.AluOpType.mult
    Add = mybir.AluOpType.add

    # weights pack: [0:64,0:66] A_lhsT (Wk^TWq fused | Wq^T bk | 0),
    # [0:66,66] b66' = [Wk^T(bq - Wq bv) ; bk.(bq - Wq bv)... see host ; 1]
    wpk = wp.tile([128, 67], F32R, tag="wpk")
    nc.sync.dma_start(wpk[:], aps["wpk"][:])
    # bf16 pack: identity for PE transposes + Wv^T (DMA deferred below --
    # it is not needed until the first boundary, keep it off the startup
    # dispatch queue)
    auxb = wp.tile([128, 192], BF16, tag="auxb")
    alb = wpk[0:64, 0:66]
    b66 = wpk[0:66, 66:67].bitcast(F32)
    id128 = auxb[:, 0:128]
    wvt = auxb[0:64, 128:192]

    # PE warm-up on the freshly loaded weights (results unread): p-state
    # ramp completes during the input DMA waits
    wps = psE.tile([128, NQ], F32, tag="e")
    for w in range(5):
        nc.tensor.matmul(wps[0:64, 0:64], wpk[0:64, 0:64],
                         wpk[0:64, 0:64], start=True, stop=True)

    def prologue(s, chunked=False):
        """Input DMAs for slot s.  chunked=True splits the big loads into
        512-col pieces so the very first energy matmul starts ASAP."""
        nk = SLOT_NK[s]
        mt = nk // 128
        xk = sb.tile([66, nk], F32R, tag="xk")
        xq = sb.tile([64, NQ], F32R, tag="xq")
        if chunked:
            # startup critical path: first qk half + first energy m-tiles
            # need only xq[:, 0:512] and xk[:, 0:512] -- dispatch those two
            # first, then the rest
            nc.sync.dma_start(xq[:, 0:HQ], aps[f"xq{s}"][:, 0:HQ])
            nc.sync.dma_start(xk[:, 0:HQ], aps[f"xk{s}"][:, 0:HQ])
            nc.sync.dma_start(xq[:, HQ:NQ], aps[f"xq{s}"][:, HQ:NQ])
        else:
            nc.sync.dma_start(xq[:], aps[f"xq{s}"][:])
        xkt = sb.tile([128, mt * 65], BF16, tag="xkt")
        if chunked:
            off = HQ
            xkt_off = 0
            while off < nk:
                w = min(512, nk - off)
                nc.sync.dma_start(xk[:, off:off + w],
                                  aps[f"xk{s}"][:, off:off + w])
                pw = min(4 * 65, mt * 65 - xkt_off)
                if pw > 0:
                    nc.sync.dma_start(
                        xkt[:, xkt_off:xkt_off + pw],
                        aps[f"xkt{s}"][:, xkt_off:xkt_off + pw])
                    xkt_off += pw
                off += w
            if xkt_off < mt * 65:
                nc.sync.dma_start(xkt[:, xkt_off:],
                                  aps[f"xkt{s}"][:, xkt_off:])
        else:
            nc.sync.dma_start(xk[:], aps[f"xk{s}"][:])
            nc.sync.dma_start(xkt[:], aps[f"xkt{s}"][:])
        xkt3 = xkt[:].rearrange("p (t c) -> p t c", c=65)
        qag = sb.tile([66, NQ], F32R, tag="qag")
        return dict(s=s, nk=nk, mt=mt, xk=xk, xq=xq, xkt3=xkt3, qag=qag)

    def qk_proj(st):
        """Fused qk projection, two 512-query halves through the shared
        1-bank transient slot; the bias-add rides ACT (Identity + bias AP)."""
        for h in (0, 1):
            qps = psO.tile([66, HQ], F32, tag="o")
            nc.tensor.matmul(qps[:], alb, st["xq"][:, h * HQ:(h + 1) * HQ],
                             start=True, stop=True)
            nc.vector.tensor_scalar_add(st["qag"][:, h * HQ:(h + 1) * HQ],
                                        qps[:], b66)

    def energy_group(st, h, tlist):
        """len(tlist) 512-query energy m-tiles into one ring tile.
        Priority boost: the PE must never run att@v backlog ahead of the
        energy that feeds the next exp."""
        eps = psE.tile([128, NQ], F32, tag="e")
        with tc.high_priority(offset=100000):
            for j, t in enumerate(tlist):
                nc.tensor.matmul(eps[:, HQ * j:HQ * (j + 1)],
                                 st["xk"][:, t * 128:(t + 1) * 128],
                                 st["qag"][:, h * HQ:(h + 1) * HQ],
                                 start=True, stop=True)
        return eps

    def attv_group(st, g, h, tlist, eps, use_dve, boost=False):
        wdt = HQ * len(tlist)
        esl = eps[:, 0:wdt]
        if use_dve:
            exi = expp.tile([128, NQ], I16, tag="ex")
            nc.vector.tensor_scalar(exi[:, 0:wdt], esl, SCH_A, SCH_B,
                                    Mult, Add)
            ex = exi[:, 0:wdt].bitcast(BF16)
        else:
            exf = expp.tile([128, NQ], BF16, tag="ex")
            nc.scalar.activation(exf[:, 0:wdt], esl, Exp)
            ex = exf[:, 0:wdt]
        mt = st["mt"]
        off = 50000 if boost else 0
        with tc.high_priority(offset=off):
            first = None
            for j, t in enumerate(tlist):
                for qc in range(4):
                    # one accumulation chain per PSUM bank: start=True zeroes
                    # the ENTIRE bank (HW-verified), so only the very first
                    # matmul of the half starts, and the other t=0 chunk
                    # matmuls get explicit scheduler edges to run after it
                    mm = nc.tensor.matmul(
                        g[:, 65 * qc:65 * qc + 65],
                        ex[:, HQ * j + 128 * qc:HQ * j + 128 * (qc + 1)],
                        st["xkt3"][:, t, :],
                        start=(t == 0 and qc == 0),
                        stop=(t == mt - 1 and qc == 3),
                        skip_group_check=True)
                    if t == 0 and qc == 0:
                        first = mm
                    elif t == 0:
                        add_dep_helper(mm.ins, first.ins, sync=False,
                                       reason="bank-start ordering")

    def boundary_half(st, g, h, split_tail=False):
        """Half tail: per-partition normalize, PE transpose, Wv matmul,
        residual add, DMA out.  Normalize rides DVE (one broadcast op),
        the T evacuation rides ACT.  split_tail (very last half only):
        run two query-pair chains in parallel, the second borrowing the
        then-free G bank, to shorten the end-of-kernel serial chain."""
        s, xq = st["s"], st["xq"]
        # ONE evacuation op releases the G bank (the next half's attv WARs
        # on it); reciprocal + normalize then run on the idle GPSIMD in
        # SBUF-land, off the exp-critical DVE/ACT queues
        gall = sbt.tile([128, 260], F32, tag="gall")
        nc.vector.tensor_copy(gall[:], g[:, 0:260])
        ga3 = gall[:].rearrange("p (c k) -> p c k", k=65)
        rec4 = sbt.tile([128, 4], F32, tag="rec4")
        nc.vector.reciprocal(rec4[:].unsqueeze(2), ga3[:, :, 64:65])
        gts = sbt.tile([128, 4 * 64], BF16, tag="gts")
        gt3 = gts[:].rearrange("p (c k) -> p c k", k=64)
        pairs = [(0, 4)] if not split_tail else [(0, 2), (2, 4)]
        for pi, (q0, q1) in enumerate(pairs):
            nc.gpsimd.tensor_tensor(
                gt3[:, q0:q1, :], ga3[:, q0:q1, 0:64],
                rec4[:, q0:q1].unsqueeze(2).broadcast_to(
                    [128, q1 - q0, 64]), Mult)
            w = 128 * (q1 - q0)
            if pi == 0:
                tps = psO.tile([64, w], BF16, tag="o", name="tps")
            else:
                tps = psG.tile([64, w], BF16, tag="g", name="tpsg")
            for qc in range(q0, q1):
                nc.tensor.transpose(tps[:, 128 * (qc - q0):128 *
                                        (qc - q0 + 1)],
                                    gts[:, 64 * qc:64 * (qc + 1)], id128)
            tsb = sbt.tile([64, w], BF16, tag="tsb", name="tsb")
            nc.scalar.activation(tsb[:], tps[:], Copy)
            if pi == 0:
                vps = psO.tile([64, w], F32, tag="o", name="vps")
            else:
                vps = psG.tile([64, w], F32, tag="g", name="vpsg")
            nc.tensor.matmul(vps[:], wvt, tsb[:], start=True, stop=True)
            fin = sbt.tile([64, w], F32, tag="fin", name="fin")
            nc.vector.tensor_tensor(
                fin[:], vps[:],
                xq[:, h * HQ + 128 * q0:h * HQ + 128 * q1], Add)
            nc.sync.dma_start(
                aps["o"][s][:, h * HQ + 128 * q0:h * HQ + 128 * q1],
                fin[:])

    # One continuous global stream of exp groups across ALL tasks; smallest
    # slot first so the exp stream warms up quickly
    order = [7, 3, 4, 0, 1, 2, 5, 6, 8] * reps
    n = len(order)
    sts = {0: prologue(order[0], chunked=True)}
    qk_proj(sts[0])
    if n > 1:
        sts[1] = prologue(order[1])
        qk_proj(sts[1])
    nc.sync.dma_start(auxb[:], aps["auxb"][:])

    # global group list: (task idx, half, tlist)
    groups = []
    for idx in range(n):
        mt = SLOT_NK[order[idx]] // 128
        for h in (0, 1):
            tl = [[t, t + 1] for t in range(0, mt - 1, 2)]
            if mt % 2:
                tl.append([mt - 1])
            for tlist in tl:
                groups.append((idx, h, tlist))

    gacc = {}
    eg = {}
    full_seen = 0
    odd_seen = 0
    for gg in (0, 1):
        gi, gh, gtl = groups[gg]
        eg[gg] = energy_group(sts[gi], gh, gtl)
    for G, (idx, h, tlist) in enumerate(groups):
        if h == 0 and tlist[0] == 0 and idx + 2 < n:
            sts[idx + 2] = prologue(order[idx + 2])
        st = sts[idx]
        if tlist[0] == 0:
            gacc[idx] = psG.tile([128, 512], F32, tag="g", name="gacc")
        if len(tlist) == 2:
            use_dve = (int((full_seen + 1) * DVE_SHARE) >
                       int(full_seen * DVE_SHARE))
            full_seen += 1
        else:
            use_dve = bool(odd_seen % 2)
            odd_seen += 1
        attv_group(st, gacc[idx], h, tlist, eg.pop(G), use_dve,
                   boost=(idx == n - 1))
        if G + 2 < len(groups):
            ni, nh, ntl = groups[G + 2]
            eg[G + 2] = energy_group(sts[ni], nh, ntl)
        if tlist[-1] == st["mt"] - 1:
            boundary_half(st, gacc[idx], h,
                          split_tail=(idx == n - 1 and h == 1))
            if h == 1:
                gacc.pop(idx)
                sts.pop(idx)
                if idx + 2 < n:
                    qk_proj(sts[idx + 2])


_CACHE = {}


def _build(reps):
    if reps in _CACHE:
        return _CACHE[reps]
    nc = bacc.Bacc("TRN2", target_bir_lowering=False, debug=False,
                   enable_asserts=True)
    aps = {}
    for s, nk in enumerate(SLOT_NK):
        aps[f"xk{s}"] = nc.dram_tensor(f"xk{s}", [66, nk], F32R,
                                       kind="ExternalInput").ap()
        aps[f"xq{s}"] = nc.dram_tensor(f"xq{s}", [64, NQ], F32R,
                                       kind="ExternalInput").ap()
        aps[f"xkt{s}"] = nc.dram_tensor(f"xkt{s}", [128, (nk // 128) * 65],
                                        BF16, kind="ExternalInput").ap()
    aps["wpk"] = nc.dram_tensor("wpk", [128, 67], F32R,
                                kind="ExternalInput").ap()
    aps["auxb"] = nc.dram_tensor("auxb", [128, 192], BF16,
                                 kind="ExternalInput").ap()
    aps["o"] = nc.dram_tensor("o", [9, 64, NQ], F32, kind="ExternalOutput").ap()

    with tile.TileContext(nc) as tc:
        with ExitStack() as ctx:
            _emit(nc, tc, ctx, aps, reps)
    nc.compile()
    _CACHE[reps] = nc
    return nc


def _host_inputs(x, Wq, bq, Wk, bk, Wv, bv):
    x = np.asarray(x, np.float32)
    Wq = np.asarray(Wq, np.float32)
    Wk = np.asarray(Wk, np.float32)
    Wv = np.asarray(Wv, np.float32)
    bq = np.asarray(bq, np.float32)
    bk = np.asarray(bk, np.float32)
    bv = np.asarray(bv, np.float32)

    wpk = np.zeros((128, 67), np.float32)
    wpk[0:64, 0:64] = Wq.T @ Wk     # A_lhsT = (Wk^T Wq)^T
    wpk[0:64, 64] = Wq.T @ bk
    # query input is xq' = xq + bv (residual add applies bv for free);
    # correct the biases for the Wq/Wk projections accordingly
    bqv = bq - Wq @ bv
    wpk[0:64, 66] = Wk.T @ bqv
    wpk[64, 66] = bk @ bqv
    wpk[65, 66] = 1.0               # turns the mask channel on
    auxb = np.zeros((128, 192), np.float32)
    auxb[:, 0:128] = np.eye(128, dtype=np.float32)
    auxb[0:64, 128:192] = Wv.T
    auxb = auxb.astype(ml_dtypes.bfloat16)
    shared = {"wpk": wpk, "auxb": auxb}
    in_maps = []
    for c in range(N_CORES):
        m = dict(shared)
        for s, (b, i, j, z) in enumerate(TASKS[c]):
            nk_slot = SLOT_NK[s]
            sx, dx = _win(i)
            sy, dy = _win(j)
            sz, dz = _win_z(z)
            win = x[b, :, sx:sx + dx, sy:sy + dy, sz:sz + dz]
            nk = dx * dy * dz
            xkb = np.zeros((66, nk_slot), np.float32)
            xkb[0:64, :nk] = win.reshape(64, nk)
            xkb[64, :nk] = 1.0
            xkb[65, nk:] = MASK_NEG
            m[f"xk{s}"] = xkb
            mt = nk_slot // 128
            # xkt: (128 keys, m-tile, 64 ch + ones col), bf16
            xkt = np.zeros((128, mt, 65), np.float32)
            xkt[:, :, 0:64] = xkb[0:64].reshape(64, mt, 128).transpose(2, 1, 0)
            xkt[:, :, 64] = 1.0
            # padded keys: zero data so their (vanishing) weights stay clean
            if nk < nk_slot:
                flat = xkt.transpose(1, 0, 2).reshape(mt * 128, 65)
                flat[nk:, 0:64] = 0.0
                xkt = flat.reshape(mt, 128, 65).transpose(1, 0, 2)
            m[f"xkt{s}"] = np.ascontiguousarray(
                xkt.reshape(128, mt * 65)).astype(ml_dtypes.bfloat16)
            m[f"xq{s}"] = np.ascontiguousarray(
                win[:, 0:16, 0:16, 0:4].reshape(64, NQ)) + bv[:, None]
        in_maps.append(m)
    return in_maps


def _scatter(results):
    out = np.empty((B, C, H, W, T), np.float32)
    for c in range(N_CORES):
        o = results[c]["o"]
        for s, (b, i, j, z) in enumerate(TASKS[c]):
            sx, _ = _win(i)
            sy, _ = _win(j)
            sz, _ = _win_z(z)
            blk = o[s].reshape(64, 16, 16, 4)
            out[b, :, sx:sx + 16, sy:sy + 16, sz:sz + 4] = blk
    return out


def _ensure_axon():
    # The axon PJRT plugin is registered by sitecustomize at interpreter
    # start; if a caller pinned JAX_PLATFORMS=cpu before jax init, try to
    # re-enable the axon backend (run_bass_via_pjrt needs 8 trn2 devices).
    import jax

    try:
        if any(d.platform == "axon" for d in jax.devices()):
            return
    except Exception:
        pass
    try:
        jax.config.update("jax_platforms", "axon,cpu")
        jax.extend.backend.clear_backends()
    except Exception:
        pass


def run(x, Wq, bq, Wk, bk, Wv, bv, reps=1):
    _ensure_axon()
    nc = _build(reps)
    in_maps = _host_inputs(x, Wq, bq, Wk, bk, Wv, bv)
    res = run_bass_kernel_spmd(nc, in_maps, core_ids=list(range(N_CORES)))
    return _scatter(res.results), res


def kernel(x, Wq, bq, Wk, bk, Wv, bv):
    out, _ = run(x, Wq, bq, Wk, bk, Wv, bv,
                 reps=int(os.environ.get("KREP", "1")))
    return out


# revision 95
# speedup vs baseline: 1.0047x; 1.0047x over previous
"""Trainium2 Bass kernel for nn_Block_self_attention_inter_intra_3D.

Math: the reference loops 36 overlapping windows (i,j in 0..2, z in 0..3) of a
(2,64,48,48,16) volume, runs channel-projected position attention inside each
window (reading the ORIGINAL x), and writes results back last-write-wins.
Each window "owns" exactly its local [0:16,0:16,0:4] sub-box (1024 query
positions); keys are all N window positions (N in {3456,2304,1536,1024}).

Sharding: 72 (window,batch) tasks -> 8 cores x 9 uniform slots
(3x N=3456, 4x N=2304, 2x N=1536; the two N=1024 tasks are padded to 1536
with a -80 additive key mask folded in as an extra contraction channel).

Per-task device pipeline, processed in two 512-query halves.  Both q- and
k-projections are fused into ONE projection of the queries, and the
v-projection is factored through the raw inputs (G-trick):
  qk_aug = A^T xq' + c'     A = Wk^T Wq; xq' = xq + bv (host); c' absorbs
                            bq/bk/bv so the residual add also applies bv
  energyT[m,n] = xk66[:,m]^T qk_aug[:,n]      (f32r, 512-row matmuls)
  ex = exp(energyT)         split across TWO engines: ACT runs exact exp
                            (bf16 out); DVE runs Schraudolph exp -- one
                            tensor_scalar producing int16 bf16-bit-patterns
  G[q, c]  (query-partition layout, 4x 128-query chunks x 65 cols/bank):
      G[:, qc*65+c] += ex_chunk^T @ xkt   (bf16, 65 rows per matmul -- this
                            orientation costs 65 PE cycles/chunk instead of
                            512, cutting att@v PE time by ~8x)
      col 64 of each chunk = sum(ex) via a ones-column in xkt
  normalize per partition:  gT = G * (1/sumexp)[q]  (scale sits on the
                            PARTITION axis, so it is a cheap scalar-AP op)
  transpose gT (PE, bf16, 4x 128x64 blocks) -> T[c, q]
  out = Wv^T T + xq'        (bf16 Wv matmul + DVE add, residual+bv via xq')

PSUM (8 banks): 3 x (128,1024) energy ring tiles = 6 banks + 1 bank G
accumulator + 1 bank shared transient slot (qk halves / transpose target /
Wv out).  Hardware constraints discovered on the way (each verified with a
micro-kernel on the real device):
  - start=True zeroes the ENTIRE psum bank -> exactly one accumulation
    chain per bank, one start per half, explicit scheduler edges so the
    starting matmul runs before the other t=0 chunk matmuls;
  - the dep tracker treats start=True matmuls as whole-tile accesses, so
    ring slots must be separate pool tiles, not offsets into one tile;
  - GPSIMD cannot touch PSUM at all, so every PSUM-side element-wise op
    lives on ACT or DVE: normalize/recip/residual-add on DVE, T-evac and
    qk bias-add (Identity + bias AP) on ACT, balanced against the exp
    split (DVE 76 of 174 full groups, Bresenham-interleaved so the ring
    always has both consumers draining in parallel).
The last task's final half splits its tail into two query-pair chains
(the second borrowing the then-free G bank) to shorten the end-of-kernel
serial chain.  Sim/HW: 205.7us (prev session baseline) -> 155.6us.
"""

import os
import sys

sys.path.insert(0, "/opt/trn_rl_repo")

from contextlib import ExitStack

import ml_dtypes
import numpy as np

import concourse.bacc as bacc
import concourse.mybir as mybir
import concourse.tile as tile
from concourse.bass_utils import run_bass_kernel_spmd
from concourse.tile_rust import add_dep_helper

F32 = mybir.dt.float32
F32R = mybir.dt.float32r
I16 = mybir.dt.int16
BF16 = mybir.dt.bfloat16

N_CORES = 8
NQ = 1024
HQ = 512
SLOT_NK = [3456, 3456, 3456, 2304, 2304, 2304, 2304, 1536, 1536]
# mask energy: exp(-80) ~ 1.8e-35 vanishes in both the exact exp and the
# Schraudolph path (whose int16 arithmetic would overflow around -1e3)
MASK_NEG = -80.0

# Schraudolph exp on DVE: exp(x) ~ bitcast_bf16(int16(x * A + B)), computed
# in f32 with a round-to-nearest int16 store (HW-verified).  B folds a -7.29
# shift that centers the (1+f)/2^f interpolation error (3.45% RMS).
SCH_A = 128.0 / float(np.log(2.0))          # 2^7 / ln 2
SCH_B = float(127 * (1 << 7)) - 477500.0 / (1 << 16)
# of the 174 full-width exp groups DVE takes 76 (see module docstring)
DVE_SHARE = 66.0 / 174.0

B, C, H, W, T = 2, 64, 48, 48, 16


def _win(i):
    s = 16 * i
    return s, min(s + 24, 48) - s


def _win_z(z):
    s = 4 * z
    return s, min(s + 6, 16) - s


def _task_lists():
    t3456 = [(b, i, j, z) for b in (0, 1) for i in (0, 1) for j in (0, 1)
             for z in (0, 1, 2)]
    t2304 = ([(b, i, j, 3) for b in (0, 1) for i in (0, 1) for j in (0, 1)] +
             [(b, i, 2, z) for b in (0, 1) for i in (0, 1) for z in (0, 1, 2)] +
             [(b, 2, j, z) for b in (0, 1) for j in (0, 1) for z in (0, 1, 2)])
    t1536 = ([(b, i, 2, 3) for b in (0, 1) for i in (0, 1)] +
             [(b, 2, j, 3) for b in (0, 1) for j in (0, 1)] +
             [(b, 2, 2, z) for b in (0, 1) for z in (0, 1, 2)])
    t1024 = [(b, 2, 2, 3) for b in (0, 1)]
    assert len(t3456) == 24 and len(t2304) == 32
    assert len(t1536) == 14 and len(t1024) == 2
    tail = t1536 + t1024
    per_core = []
    for c in range(N_CORES):
        per_core.append(t3456[3 * c:3 * c + 3] + t2304[4 * c:4 * c + 4] +
                        tail[2 * c:2 * c + 2])
    return per_core


TASKS = _task_lists()


def _emit(nc, tc, ctx, aps, reps):
    wp = ctx.enter_context(tc.tile_pool(name="wp", bufs=1))
    sb = ctx.enter_context(tc.tile_pool(name="sb", bufs=3))
    sbt = ctx.enter_context(tc.tile_pool(name="sbt", bufs=5))
    expp = ctx.enter_context(tc.tile_pool(name="expp", bufs=10))
    # PSUM: E ring 3 x (128,1024) tiles = 6 banks + G 1 bank + transient
    # slot 1 bank = 8.  Separate ring TILES (not offsets in one tile):
    # start=True matmuls act on the whole bank, and the dep tracker
    # accordingly treats them as whole-tile accesses -- one shared tile
    # would serialize the entire stream.
    psE = ctx.enter_context(tc.tile_pool(name="psE", bufs=3, space="PSUM"))
    psG = ctx.enter_context(tc.tile_pool(name="psG", bufs=1, space="PSUM"))
    psO = ctx.enter_context(tc.tile_pool(name="psO", bufs=1, space="PSUM"))

    Exp = mybir.ActivationFunctionType.Exp
    Copy = mybir.ActivationFunctionType.Copy
    Ident = mybir.ActivationFunctionType.Identity
    Mult = mybir.AluOpType.mult
    Add = mybir.AluOpType.add

    # weights pack: [0:64,0:66] A_lhsT (Wk^TWq fused | Wq^T bk | 0),
    # [0:66,66] b66' = [Wk^T(bq - Wq bv) ; bk.(bq - Wq bv)... see host ; 1]
    wpk = wp.tile([128, 67], F32R, tag="wpk")
    nc.sync.dma_start(wpk[:], aps["wpk"][:])
    # bf16 pack: identity for PE transposes + Wv^T (DMA deferred below --
    # it is not needed until the first boundary, keep it off the startup
    # dispatch queue)
    auxb = wp.tile([128, 192], BF16, tag="auxb")
    alb = wpk[0:64, 0:66]
    b66 = wpk[0:66, 66:67].bitcast(F32)
    id128 = auxb[:, 0:128]
    wvt = auxb[0:64, 128:192]

    # PE warm-up on the freshly loaded weights (results unread): p-state
    # ramp completes during the input DMA waits
    wps = psE.tile([128, NQ], F32, tag="e")
    for w in range(5):
        nc.tensor.matmul(wps[0:64, 0:64], wpk[0:64, 0:64],
                         wpk[0:64, 0:64], start=True, stop=True)

    def prologue(s, chunked=False):
        """Input DMAs for slot s.  chunked=True splits the big loads into
        512-col pieces so the very first energy matmul starts ASAP."""
        nk = SLOT_NK[s]
        mt = nk // 128
        xk = sb.tile([66, nk], F32R, tag="xk")
        xq = sb.tile([64, NQ], F32R, tag="xq")
        if chunked:
            # startup critical path: first qk half + first energy m-tiles
            # need only xq[:, 0:512] and xk[:, 0:512] -- dispatch those two
            # first, then the rest
            nc.sync.dma_start(xq[:, 0:HQ], aps[f"xq{s}"][:, 0:HQ])
            nc.sync.dma_start(xk[:, 0:HQ], aps[f"xk{s}"][:, 0:HQ])
            nc.sync.dma_start(xq[:, HQ:NQ], aps[f"xq{s}"][:, HQ:NQ])
        else:
            nc.sync.dma_start(xq[:], aps[f"xq{s}"][:])
        xkt = sb.tile([128, mt * 65], BF16, tag="xkt")
        if chunked:
            off = HQ
            xkt_off = 0
            while off < nk:
                w = min(512, nk - off)
                nc.sync.dma_start(xk[:, off:off + w],
                                  aps[f"xk{s}"][:, off:off + w])
                pw = min(4 * 65, mt * 65 - xkt_off)
                if pw > 0:
                    nc.sync.dma_start(
                        xkt[:, xkt_off:xkt_off + pw],
                        aps[f"xkt{s}"][:, xkt_off:xkt_off + pw])
                    xkt_off += pw
                off += w
            if xkt_off < mt * 65:
                nc.sync.dma_start(xkt[:, xkt_off:],
                                  aps[f"xkt{s}"][:, xkt_off:])
        else:
            nc.sync.dma_start(xk[:], aps[f"xk{s}"][:])
            nc.sync.dma_start(xkt[:], aps[f"xkt{s}"][:])
        xkt3 = xkt[:].rearrange("p (t c) -> p t c", c=65)
        qag = sb.tile([66, NQ], F32R, tag="qag")
        return dict(s=s, nk=nk, mt=mt, xk=xk, xq=xq, xkt3=xkt3, qag=qag)

    def qk_proj(st):
        """Fused qk projection, two 512-query halves through the shared
        1-bank transient slot; the bias-add rides ACT (Identity + bias AP)."""
        for h in (0, 1):
            qps = psO.tile([66, HQ], F32, tag="o")
            nc.tensor.matmul(qps[:], alb, st["xq"][:, h * HQ:(h + 1) * HQ],
                             start=True, stop=True)
            nc.vector.tensor_scalar_add(st["qag"][:, h * HQ:(h + 1) * HQ],
                                        qps[:], b66)

    def energy_group(st, h, tlist):
        """len(tlist) 512-query energy m-tiles into one ring tile.
        Priority boost: the PE must never run att@v backlog ahead of the
        energy that feeds the next exp."""
        eps = psE.tile([128, NQ], F32, tag="e")
        with tc.high_priority(offset=100000):
            for j, t in enumerate(tlist):
                nc.tensor.matmul(eps[:, HQ * j:HQ * (j + 1)],
                                 st["xk"][:, t * 128:(t + 1) * 128],
                                 st["qag"][:, h * HQ:(h + 1) * HQ],
                                 start=True, stop=True)
        return eps

    def attv_group(st, g, h, tlist, eps, use_dve, boost=False):
        wdt = HQ * len(tlist)
        esl = eps[:, 0:wdt]
        if use_dve:
            exi = expp.tile([128, NQ], I16, tag="ex")
            nc.vector.tensor_scalar(exi[:, 0:wdt], esl, SCH_A, SCH_B,
                                    Mult, Add)
            ex = exi[:, 0:wdt].bitcast(BF16)
        else:
            exf = expp.tile([128, NQ], BF16, tag="ex")
            nc.scalar.activation(exf[:, 0:wdt], esl, Exp)
            ex = exf[:, 0:wdt]
        mt = st["mt"]
        off = 50000 if boost else 0
        with tc.high_priority(offset=off):
            first = None
            for j, t in enumerate(tlist):
                for qc in range(4):
                    # one accumulation chain per PSUM bank: start=True zeroes
                    # the ENTIRE bank (HW-verified), so only the very first
                    # matmul of the half starts, and the other t=0 chunk
                    # matmuls get explicit scheduler edges to run after it
                    mm = nc.tensor.matmul(
                        g[:, 65 * qc:65 * qc + 65],
                        ex[:, HQ * j + 128 * qc:HQ * j + 128 * (qc + 1)],
                        st["xkt3"][:, t, :],
                        start=(t == 0 and qc == 0),
                        stop=(t == mt - 1 and qc == 3),
                        skip_group_check=True)
                    if t == 0 and qc == 0:
                        first = mm
                    elif t == 0:
                        add_dep_helper(mm.ins, first.ins, sync=False,
                                       reason="bank-start ordering")

    def boundary_half(st, g, h, split_tail=False):
        """Half tail: per-partition normalize, PE transpose, Wv matmul,
        residual add, DMA out.  Normalize rides DVE (one broadcast op),
        the T evacuation rides ACT.  split_tail (very last half only):
        run two query-pair chains in parallel, the second borrowing the
        then-free G bank, to shorten the end-of-kernel serial chain."""
        s, xq = st["s"], st["xq"]
        # ONE evacuation op releases the G bank (the next half's attv WARs
        # on it); reciprocal + normalize then run on the idle GPSIMD in
        # SBUF-land, off the exp-critical DVE/ACT queues
        gall = sbt.tile([128, 260], F32, tag="gall")
        nc.vector.tensor_copy(gall[:], g[:, 0:260])
        ga3 = gall[:].rearrange("p (c k) -> p c k", k=65)
        rec4 = sbt.tile([128, 4], F32, tag="rec4")
        nc.vector.reciprocal(rec4[:].unsqueeze(2), ga3[:, :, 64:65])
        gts = sbt.tile([128, 4 * 64], BF16, tag="gts")
        gt3 = gts[:].rearrange("p (c k) -> p c k", k=64)
        pairs = [(0, 4)] if not split_tail else [(0, 2), (2, 4)]
        for pi, (q0, q1) in enumerate(pairs):
            nc.gpsimd.tensor_tensor(
                gt3[:, q0:q1, :], ga3[:, q0:q1, 0:64],
                rec4[:, q0:q1].unsqueeze(2).broadcast_to(
                    [128, q1 - q0, 64]), Mult)
            w = 128 * (q1 - q0)
            if pi == 0:
                tps = psO.tile([64, w], BF16, tag="o", name="tps")
            else:
                tps = psG.tile([64, w], BF16, tag="g", name="tpsg")
            for qc in range(q0, q1):
                nc.tensor.transpose(tps[:, 128 * (qc - q0):128 *
                                        (qc - q0 + 1)],
                                    gts[:, 64 * qc:64 * (qc + 1)], id128)
            tsb = sbt.tile([64, w], BF16, tag="tsb", name="tsb")
            nc.scalar.activation(tsb[:], tps[:], Copy)
            if pi == 0:
                vps = psO.tile([64, w], F32, tag="o", name="vps")
            else:
                vps = psG.tile([64, w], F32, tag="g", name="vpsg")
            nc.tensor.matmul(vps[:], wvt, tsb[:], start=True, stop=True)
            fin = sbt.tile([64, w], F32, tag="fin", name="fin")
            nc.vector.tensor_tensor(
                fin[:], vps[:],
                xq[:, h * HQ + 128 * q0:h * HQ + 128 * q1], Add)
            nc.sync.dma_start(
                aps["o"][s][:, h * HQ + 128 * q0:h * HQ + 128 * q1],
                fin[:])

    # One continuous global stream of exp groups across ALL tasks; smallest
    # slot first so the exp stream warms up quickly
    order = [7, 0, 3, 1, 4, 2, 5, 6, 8] * reps
    n = len(order)
    sts = {0: prologue(order[0], chunked=True)}
    qk_proj(sts[0])
    if n > 1:
        sts[1] = prologue(order[1])
        qk_proj(sts[1])
    nc.sync.dma_start(auxb[:], aps["auxb"][:])

    # global group list: (task idx, half, tlist)
    groups = []
    for idx in range(n):
        mt = SLOT_NK[order[idx]] // 128
        for h in (0, 1):
            tl = [[t, t + 1] for t in range(0, mt - 1, 2)]
            if mt % 2:
                tl.append([mt - 1])
            for tlist in tl:
                groups.append((idx, h, tlist))

    gacc = {}
    eg = {}
    full_seen = 0
    odd_seen = 0
    for gg in (0, 1):
        gi, gh, gtl = groups[gg]
        eg[gg] = energy_group(sts[gi], gh, gtl)
    for G, (idx, h, tlist) in enumerate(groups):
        if h == 0 and tlist[0] == 0 and idx + 2 < n:
            sts[idx + 2] = prologue(order[idx + 2])
        st = sts[idx]
        if tlist[0] == 0:
            gacc[idx] = psG.tile([128, 512], F32, tag="g", name="gacc")
        if len(tlist) == 2:
            use_dve = (int((full_seen + 1) * DVE_SHARE) >
                       int(full_seen * DVE_SHARE))
            if G == 1:
                # startup: DVE is idle; don't serialize the first two
                # groups on ACT while the ring fills
                use_dve = True
            full_seen += 1
        else:
            use_dve = bool(odd_seen % 2)
            odd_seen += 1
        attv_group(st, gacc[idx], h, tlist, eg.pop(G), use_dve,
                   boost=(idx == n - 1))
        if G + 2 < len(groups):
            ni, nh, ntl = groups[G + 2]
            eg[G + 2] = energy_group(sts[ni], nh, ntl)
        if tlist[-1] == st["mt"] - 1:
            boundary_half(st, gacc[idx], h,
                          split_tail=(idx == n - 1 and h == 1))
            if h == 1:
                gacc.pop(idx)
                sts.pop(idx)
                if idx + 2 < n:
                    qk_proj(sts[idx + 2])


_CACHE = {}


def _build(reps):
    if reps in _CACHE:
        return _CACHE[reps]
    nc = bacc.Bacc("TRN2", target_bir_lowering=False, debug=False,
                   enable_asserts=True)
    aps = {}
    for s, nk in enumerate(SLOT_NK):
        aps[f"xk{s}"] = nc.dram_tensor(f"xk{s}", [66, nk], F32R,
                                       kind="ExternalInput").ap()
        aps[f"xq{s}"] = nc.dram_tensor(f"xq{s}", [64, NQ], F32R,
                                       kind="ExternalInput").ap()
        aps[f"xkt{s}"] = nc.dram_tensor(f"xkt{s}", [128, (nk // 128) * 65],
                                        BF16, kind="ExternalInput").ap()
    aps["wpk"] = nc.dram_tensor("wpk", [128, 67], F32R,
                                kind="ExternalInput").ap()
    aps["auxb"] = nc.dram_tensor("auxb", [128, 192], BF16,
                                 kind="ExternalInput").ap()
    aps["o"] = nc.dram_tensor("o", [9, 64, NQ], F32, kind="ExternalOutput").ap()

    with tile.TileContext(nc) as tc:
        with ExitStack() as ctx:
            _emit(nc, tc, ctx, aps, reps)
    nc.compile()
    _CACHE[reps] = nc
    return nc


def _host_inputs(x, Wq, bq, Wk, bk, Wv, bv):
    x = np.asarray(x, np.float32)
    Wq = np.asarray(Wq, np.float32)
    Wk = np.asarray(Wk, np.float32)
    Wv = np.asarray(Wv, np.float32)
    bq = np.asarray(bq, np.float32)
    bk = np.asarray(bk, np.float32)
    bv = np.asarray(bv, np.float32)

    wpk = np.zeros((128, 67), np.float32)
    wpk[0:64, 0:64] = Wq.T @ Wk     # A_lhsT = (Wk^T Wq)^T
    wpk[0:64, 64] = Wq.T @ bk
    # query input is xq' = xq + bv (residual add applies bv for free);
    # correct the biases for the Wq/Wk projections accordingly
    bqv = bq - Wq @ bv
    wpk[0:64, 66] = Wk.T @ bqv
    wpk[64, 66] = bk @ bqv
    wpk[65, 66] = 1.0               # turns the mask channel on
    auxb = np.zeros((128, 192), np.float32)
    auxb[:, 0:128] = np.eye(128, dtype=np.float32)
    auxb[0:64, 128:192] = Wv.T
    auxb = auxb.astype(ml_dtypes.bfloat16)
    shared = {"wpk": wpk, "auxb": auxb}
    in_maps = []
    for c in range(N_CORES):
        m = dict(shared)
        for s, (b, i, j, z) in enumerate(TASKS[c]):
            nk_slot = SLOT_NK[s]
            sx, dx = _win(i)
            sy, dy = _win(j)
            sz, dz = _win_z(z)
            win = x[b, :, sx:sx + dx, sy:sy + dy, sz:sz + dz]
            nk = dx * dy * dz
            xkb = np.zeros((66, nk_slot), np.float32)
            xkb[0:64, :nk] = win.reshape(64, nk)
            xkb[64, :nk] = 1.0
            xkb[65, nk:] = MASK_NEG
            m[f"xk{s}"] = xkb
            mt = nk_slot // 128
            # xkt: (128 keys, m-tile, 64 ch + ones col), bf16
            xkt = np.zeros((128, mt, 65), np.float32)
            xkt[:, :, 0:64] = xkb[0:64].reshape(64, mt, 128).transpose(2, 1, 0)
            xkt[:, :, 64] = 1.0
            # padded keys: zero data so their (vanishing) weights stay clean
            if nk < nk_slot:
                flat = xkt.transpose(1, 0, 2).reshape(mt * 128, 65)
                flat[nk:, 0:64] = 0.0
                xkt = flat.reshape(mt, 128, 65).transpose(1, 0, 2)
            m[f"xkt{s}"] = np.ascontiguousarray(
                xkt.reshape(128, mt * 65)).astype(ml_dtypes.bfloat16)
            m[f"xq{s}"] = np.ascontiguousarray(
                win[:, 0:16, 0:16, 0:4].reshape(64, NQ)) + bv[:, None]
        in_maps.append(m)
    return in_maps


def _scatter(results):
    out = np.empty((B, C, H, W, T), np.float32)
    for c in range(N_CORES):
        o = results[c]["o"]
        for s, (b, i, j, z) in enumerate(TASKS[c]):
            sx, _ = _win(i)
            sy, _ = _win(j)
            sz, _ = _win_z(z)
            blk = o[s].reshape(64, 16, 16, 4)
            out[b, :, sx:sx + 16, sy:sy + 16, sz:sz + 4] = blk
    return out


def _ensure_axon():
    # The axon PJRT plugin is registered by sitecustomize at interpreter
    # start; if a caller pinned JAX_PLATFORMS=cpu before jax init, try to
    # re-enable the axon backend (run_bass_via_pjrt needs 8 trn2 devices).
    import jax

    try:
        if any(d.platform == "axon" for d in jax.devices()):
            return
    except Exception:
        pass
    try:
        jax.config.update("jax_platforms", "axon,cpu")
        jax.extend.backend.clear_backends()
    except Exception:
        pass


def run(x, Wq, bq, Wk, bk, Wv, bv, reps=1):
    _ensure_axon()
    nc = _build(reps)
    in_maps = _host_inputs(x, Wq, bq, Wk, bk, Wv, bv)
    res = run_bass_kernel_spmd(nc, in_maps, core_ids=list(range(N_CORES)))
    return _scatter(res.results), res


def kernel(x, Wq, bq, Wk, bk, Wv, bv):
    out, _ = run(x, Wq, bq, Wk, bk, Wv, bv,
                 reps=int(os.environ.get("KREP", "1")))
    return out


# revision 96
# speedup vs baseline: 1.0115x; 1.0068x over previous
"""Trainium2 Bass kernel for nn_Block_self_attention_inter_intra_3D.

Math: the reference loops 36 overlapping windows (i,j in 0..2, z in 0..3) of a
(2,64,48,48,16) volume, runs channel-projected position attention inside each
window (reading the ORIGINAL x), and writes results back last-write-wins.
Each window "owns" exactly its local [0:16,0:16,0:4] sub-box (1024 query
positions); keys are all N window positions (N in {3456,2304,1536,1024}).

Sharding: 72 (window,batch) tasks -> 8 cores x 9 uniform slots
(3x N=3456, 4x N=2304, 2x N=1536; the two N=1024 tasks are padded to 1536
with a -80 additive key mask folded in as an extra contraction channel).

Per-task device pipeline, processed in two 512-query halves.  Both q- and
k-projections are fused into ONE projection of the queries, and the
v-projection is factored through the raw inputs (G-trick):
  qk_aug = A^T xq' + c'     A = Wk^T Wq; xq' = xq + bv (host); c' absorbs
                            bq/bk/bv so the residual add also applies bv
  energyT[m,n] = xk66[:,m]^T qk_aug[:,n]      (f32r, 512-row matmuls)
  ex = exp(energyT)         split across TWO engines: ACT runs exact exp
                            (bf16 out); DVE runs Schraudolph exp -- one
                            tensor_scalar producing int16 bf16-bit-patterns
  G[q, c]  (query-partition layout, 4x 128-query chunks x 65 cols/bank):
      G[:, qc*65+c] += ex_chunk^T @ xkt   (bf16, 65 rows per matmul -- this
                            orientation costs 65 PE cycles/chunk instead of
                            512, cutting att@v PE time by ~8x)
      col 64 of each chunk = sum(ex) via a ones-column in xkt
  normalize per partition:  gT = G * (1/sumexp)[q]  (scale sits on the
                            PARTITION axis, so it is a cheap scalar-AP op)
  transpose gT (PE, bf16, 4x 128x64 blocks) -> T[c, q]
  out = Wv^T T + xq'        (bf16 Wv matmul + DVE add, residual+bv via xq')

PSUM (8 banks): 3 x (128,1024) energy ring tiles = 6 banks + 1 bank G
accumulator + 1 bank shared transient slot (qk halves / transpose target /
Wv out).  Hardware constraints discovered on the way (each verified with a
micro-kernel on the real device):
  - start=True zeroes the ENTIRE psum bank -> exactly one accumulation
    chain per bank, one start per half, explicit scheduler edges so the
    starting matmul runs before the other t=0 chunk matmuls;
  - the dep tracker treats start=True matmuls as whole-tile accesses, so
    ring slots must be separate pool tiles, not offsets into one tile;
  - GPSIMD cannot touch PSUM at all, so every PSUM-side element-wise op
    lives on ACT or DVE: normalize/recip/residual-add on DVE, T-evac and
    qk bias-add (Identity + bias AP) on ACT, balanced against the exp
    split (DVE 66 of 174 full groups, Bresenham-interleaved so the ring
    always has both consumers draining in parallel).
The last task's final half splits its tail into two query-pair chains
(the second borrowing the then-free G bank) to shorten the end-of-kernel
serial chain.  Sim/HW: 205.7us (prev session baseline) -> 155.6us.
"""

import os
import sys

sys.path.insert(0, "/opt/trn_rl_repo")

from contextlib import ExitStack

import ml_dtypes
import numpy as np

import concourse.bacc as bacc
import concourse.mybir as mybir
import concourse.tile as tile
from concourse.bass_utils import run_bass_kernel_spmd
from concourse.tile_rust import add_dep_helper

F32 = mybir.dt.float32
F32R = mybir.dt.float32r
I16 = mybir.dt.int16
BF16 = mybir.dt.bfloat16

N_CORES = 8
NQ = 1024
HQ = 512
SLOT_NK = [3456, 3456, 3456, 2304, 2304, 2304, 2304, 1536, 1536]
# mask energy: exp(-80) ~ 1.8e-35 vanishes in both the exact exp and the
# Schraudolph path (whose int16 arithmetic would overflow around -1e3)
MASK_NEG = -80.0

# Schraudolph exp on DVE: exp(x) ~ bitcast_bf16(int16(x * A + B)), computed
# in f32 with a round-to-nearest int16 store (HW-verified).  B folds a -7.29
# shift that centers the (1+f)/2^f interpolation error (3.45% RMS).
SCH_A = 128.0 / float(np.log(2.0))          # 2^7 / ln 2
SCH_B = float(127 * (1 << 7)) - 477500.0 / (1 << 16)
# of the 174 full-width exp groups DVE takes 76 (see module docstring)
DVE_SHARE = 66.0 / 174.0

B, C, H, W, T = 2, 64, 48, 48, 16


def _win(i):
    s = 16 * i
    return s, min(s + 24, 48) - s


def _win_z(z):
    s = 4 * z
    return s, min(s + 6, 16) - s


def _task_lists():
    t3456 = [(b, i, j, z) for b in (0, 1) for i in (0, 1) for j in (0, 1)
             for z in (0, 1, 2)]
    t2304 = ([(b, i, j, 3) for b in (0, 1) for i in (0, 1) for j in (0, 1)] +
             [(b, i, 2, z) for b in (0, 1) for i in (0, 1) for z in (0, 1, 2)] +
             [(b, 2, j, z) for b in (0, 1) for j in (0, 1) for z in (0, 1, 2)])
    t1536 = ([(b, i, 2, 3) for b in (0, 1) for i in (0, 1)] +
             [(b, 2, j, 3) for b in (0, 1) for j in (0, 1)] +
             [(b, 2, 2, z) for b in (0, 1) for z in (0, 1, 2)])
    t1024 = [(b, 2, 2, 3) for b in (0, 1)]
    assert len(t3456) == 24 and len(t2304) == 32
    assert len(t1536) == 14 and len(t1024) == 2
    tail = t1536 + t1024
    per_core = []
    for c in range(N_CORES):
        per_core.append(t3456[3 * c:3 * c + 3] + t2304[4 * c:4 * c + 4] +
                        tail[2 * c:2 * c + 2])
    return per_core


TASKS = _task_lists()


def _emit(nc, tc, ctx, aps, reps):
    wp = ctx.enter_context(tc.tile_pool(name="wp", bufs=1))
    sb = ctx.enter_context(tc.tile_pool(name="sb", bufs=3))
    sbt = ctx.enter_context(tc.tile_pool(name="sbt", bufs=5))
    expp = ctx.enter_context(tc.tile_pool(name="expp", bufs=10))
    # PSUM: E ring 3 x (128,1024) tiles = 6 banks + G 1 bank + transient
    # slot 1 bank = 8.  Separate ring TILES (not offsets in one tile):
    # start=True matmuls act on the whole bank, and the dep tracker
    # accordingly treats them as whole-tile accesses -- one shared tile
    # would serialize the entire stream.
    psE = ctx.enter_context(tc.tile_pool(name="psE", bufs=3, space="PSUM"))
    psG = ctx.enter_context(tc.tile_pool(name="psG", bufs=1, space="PSUM"))
    psO = ctx.enter_context(tc.tile_pool(name="psO", bufs=1, space="PSUM"))

    Exp = mybir.ActivationFunctionType.Exp
    Copy = mybir.ActivationFunctionType.Copy
    Ident = mybir.ActivationFunctionType.Identity
    Mult = mybir.AluOpType.mult
    Add = mybir.AluOpType.add

    # weights pack: [0:64,0:66] A_lhsT (Wk^TWq fused | Wq^T bk | 0),
    # [0:66,66] b66' = [Wk^T(bq - Wq bv) ; bk.(bq - Wq bv)... see host ; 1]
    wpk = wp.tile([128, 67], F32R, tag="wpk")
    nc.sync.dma_start(wpk[:], aps["wpk"][:])
    # bf16 pack: identity for PE transposes + Wv^T (DMA deferred below --
    # it is not needed until the first boundary, keep it off the startup
    # dispatch queue)
    auxb = wp.tile([128, 192], BF16, tag="auxb")
    alb = wpk[0:64, 0:66]
    b66 = wpk[0:66, 66:67].bitcast(F32)
    id128 = auxb[:, 0:128]
    wvt = auxb[0:64, 128:192]

    # PE warm-up on the freshly loaded weights (results unread): p-state
    # ramp completes during the input DMA waits
    wps = psE.tile([128, NQ], F32, tag="e")
    for w in range(5):
        nc.tensor.matmul(wps[0:64, 0:64], wpk[0:64, 0:64],
                         wpk[0:64, 0:64], start=True, stop=True)

    def prologue(s, chunked=False):
        """Input DMAs for slot s.  chunked=True splits the big loads into
        512-col pieces so the very first energy matmul starts ASAP."""
        nk = SLOT_NK[s]
        mt = nk // 128
        xk = sb.tile([66, nk], F32R, tag="xk")
        xq = sb.tile([64, NQ], F32R, tag="xq")
        if chunked:
            # startup critical path: first qk half + first energy m-tiles
            # need only xq[:, 0:512] and xk[:, 0:512] -- dispatch those two
            # first, then the rest
            nc.sync.dma_start(xq[:, 0:HQ], aps[f"xq{s}"][:, 0:HQ])
            nc.sync.dma_start(xk[:, 0:HQ], aps[f"xk{s}"][:, 0:HQ])
            nc.sync.dma_start(xq[:, HQ:NQ], aps[f"xq{s}"][:, HQ:NQ])
        else:
            nc.sync.dma_start(xq[:], aps[f"xq{s}"][:])
        xkt = sb.tile([128, mt * 65], BF16, tag="xkt")
        if chunked:
            off = HQ
            xkt_off = 0
            while off < nk:
                w = min(512, nk - off)
                nc.sync.dma_start(xk[:, off:off + w],
                                  aps[f"xk{s}"][:, off:off + w])
                pw = min(4 * 65, mt * 65 - xkt_off)
                if pw > 0:
                    nc.sync.dma_start(
                        xkt[:, xkt_off:xkt_off + pw],
                        aps[f"xkt{s}"][:, xkt_off:xkt_off + pw])
                    xkt_off += pw
                off += w
            if xkt_off < mt * 65:
                nc.sync.dma_start(xkt[:, xkt_off:],
                                  aps[f"xkt{s}"][:, xkt_off:])
        else:
            nc.sync.dma_start(xk[:], aps[f"xk{s}"][:])
            nc.sync.dma_start(xkt[:], aps[f"xkt{s}"][:])
        xkt3 = xkt[:].rearrange("p (t c) -> p t c", c=65)
        qag = sb.tile([66, NQ], F32R, tag="qag")
        return dict(s=s, nk=nk, mt=mt, xk=xk, xq=xq, xkt3=xkt3, qag=qag)

    def qk_proj(st):
        """Fused qk projection, two 512-query halves through the shared
        1-bank transient slot; the bias-add rides ACT (Identity + bias AP)."""
        for h in (0, 1):
            qps = psO.tile([66, HQ], F32, tag="o")
            nc.tensor.matmul(qps[:], alb, st["xq"][:, h * HQ:(h + 1) * HQ],
                             start=True, stop=True)
            nc.vector.tensor_scalar_add(st["qag"][:, h * HQ:(h + 1) * HQ],
                                        qps[:], b66)

    def energy_group(st, h, tlist):
        """len(tlist) 512-query energy m-tiles into one ring tile.
        Priority boost: the PE must never run att@v backlog ahead of the
        energy that feeds the next exp."""
        eps = psE.tile([128, NQ], F32, tag="e")
        with tc.high_priority(offset=100000):
            for j, t in enumerate(tlist):
                nc.tensor.matmul(eps[:, HQ * j:HQ * (j + 1)],
                                 st["xk"][:, t * 128:(t + 1) * 128],
                                 st["qag"][:, h * HQ:(h + 1) * HQ],
                                 start=True, stop=True)
        return eps

    def attv_group(st, g, h, tlist, eps, use_dve, boost=False):
        wdt = HQ * len(tlist)
        esl = eps[:, 0:wdt]
        if use_dve:
            exi = expp.tile([128, NQ], I16, tag="ex")
            nc.vector.tensor_scalar(exi[:, 0:wdt], esl, SCH_A, SCH_B,
                                    Mult, Add)
            ex = exi[:, 0:wdt].bitcast(BF16)
        else:
            exf = expp.tile([128, NQ], BF16, tag="ex")
            nc.scalar.activation(exf[:, 0:wdt], esl, Exp)
            ex = exf[:, 0:wdt]
        mt = st["mt"]
        off = 50000 if boost else 0
        with tc.high_priority(offset=off):
            first = None
            for j, t in enumerate(tlist):
                for qc in range(4):
                    # one accumulation chain per PSUM bank: start=True zeroes
                    # the ENTIRE bank (HW-verified), so only the very first
                    # matmul of the half starts, and the other t=0 chunk
                    # matmuls get explicit scheduler edges to run after it
                    mm = nc.tensor.matmul(
                        g[:, 65 * qc:65 * qc + 65],
                        ex[:, HQ * j + 128 * qc:HQ * j + 128 * (qc + 1)],
                        st["xkt3"][:, t, :],
                        start=(t == 0 and qc == 0),
                        stop=(t == mt - 1 and qc == 3),
                        skip_group_check=True)
                    if t == 0 and qc == 0:
                        first = mm
                    elif t == 0:
                        add_dep_helper(mm.ins, first.ins, sync=False,
                                       reason="bank-start ordering")

    def boundary_half(st, g, h, split_tail=False):
        """Half tail: per-partition normalize, PE transpose, Wv matmul,
        residual add, DMA out.  Normalize rides DVE (one broadcast op),
        the T evacuation rides ACT.  split_tail (very last half only):
        run two query-pair chains in parallel, the second borrowing the
        then-free G bank, to shorten the end-of-kernel serial chain."""
        s, xq = st["s"], st["xq"]
        # ONE evacuation op releases the G bank (the next half's attv WARs
        # on it); reciprocal + normalize then run on the idle GPSIMD in
        # SBUF-land, off the exp-critical DVE/ACT queues
        gall = sbt.tile([128, 260], F32, tag="gall")
        nc.vector.tensor_copy(gall[:], g[:, 0:260])
        ga3 = gall[:].rearrange("p (c k) -> p c k", k=65)
        rec4 = sbt.tile([128, 4], F32, tag="rec4")
        nc.vector.reciprocal(rec4[:].unsqueeze(2), ga3[:, :, 64:65])
        gts = sbt.tile([128, 4 * 64], BF16, tag="gts")
        gt3 = gts[:].rearrange("p (c k) -> p c k", k=64)
        pairs = [(0, 4)] if not split_tail else [(0, 2), (2, 4)]
        for pi, (q0, q1) in enumerate(pairs):
            nc.gpsimd.tensor_tensor(
                gt3[:, q0:q1, :], ga3[:, q0:q1, 0:64],
                rec4[:, q0:q1].unsqueeze(2).broadcast_to(
                    [128, q1 - q0, 64]), Mult)
            w = 128 * (q1 - q0)
            if pi == 0:
                tps = psO.tile([64, w], BF16, tag="o", name="tps")
            else:
                tps = psG.tile([64, w], BF16, tag="g", name="tpsg")
            for qc in range(q0, q1):
                nc.tensor.transpose(tps[:, 128 * (qc - q0):128 *
                                        (qc - q0 + 1)],
                                    gts[:, 64 * qc:64 * (qc + 1)], id128)
            tsb = sbt.tile([64, w], BF16, tag="tsb", name="tsb")
            nc.scalar.activation(tsb[:], tps[:], Copy)
            if pi == 0:
                vps = psO.tile([64, w], F32, tag="o", name="vps")
            else:
                vps = psG.tile([64, w], F32, tag="g", name="vpsg")
            nc.tensor.matmul(vps[:], wvt, tsb[:], start=True, stop=True)
            fin = sbt.tile([64, w], F32, tag="fin", name="fin")
            nc.vector.tensor_tensor(
                fin[:], vps[:],
                xq[:, h * HQ + 128 * q0:h * HQ + 128 * q1], Add)
            nc.sync.dma_start(
                aps["o"][s][:, h * HQ + 128 * q0:h * HQ + 128 * q1],
                fin[:])

    # One continuous global stream of exp groups across ALL tasks; smallest
    # slot first so the exp stream warms up quickly
    order = [7, 0, 3, 1, 4, 2, 5, 6, 8] * reps
    n = len(order)
    sts = {0: prologue(order[0], chunked=True)}
    qk_proj(sts[0])
    if n > 1:
        sts[1] = prologue(order[1])
        qk_proj(sts[1])
    nc.sync.dma_start(auxb[:], aps["auxb"][:])

    # global group list: (task idx, half, tlist)
    groups = []
    for idx in range(n):
        mt = SLOT_NK[order[idx]] // 128
        for h in (0, 1):
            tl = [[t, t + 1] for t in range(0, mt - 1, 2)]
            if mt % 2:
                tl.append([mt - 1])
            for tlist in tl:
                groups.append((idx, h, tlist))

    gacc = {}
    eg = {}
    full_seen = 0
    odd_seen = 0
    for gg in (0, 1):
        gi, gh, gtl = groups[gg]
        eg[gg] = energy_group(sts[gi], gh, gtl)
    for G, (idx, h, tlist) in enumerate(groups):
        if h == 0 and tlist[0] == 0 and idx + 2 < n:
            sts[idx + 2] = prologue(order[idx + 2])
        st = sts[idx]
        if tlist[0] == 0:
            gacc[idx] = psG.tile([128, 512], F32, tag="g", name="gacc")
        if len(tlist) == 2:
            use_dve = (int((full_seen + 1) * DVE_SHARE) >
                       int(full_seen * DVE_SHARE))
            full_seen += 1
        else:
            use_dve = bool(odd_seen % 2)
            odd_seen += 1
        attv_group(st, gacc[idx], h, tlist, eg.pop(G), use_dve,
                   boost=(idx == n - 1))
        if G + 2 < len(groups):
            ni, nh, ntl = groups[G + 2]
            eg[G + 2] = energy_group(sts[ni], nh, ntl)
        if tlist[-1] == st["mt"] - 1:
            boundary_half(st, gacc[idx], h,
                          split_tail=(idx == n - 1 and h == 1))
            if h == 1:
                gacc.pop(idx)
                sts.pop(idx)
                if idx + 2 < n:
                    qk_proj(sts[idx + 2])


_CACHE = {}


def _build(reps):
    if reps in _CACHE:
        return _CACHE[reps]
    nc = bacc.Bacc("TRN2", target_bir_lowering=False, debug=False,
                   enable_asserts=True)
    aps = {}
    for s, nk in enumerate(SLOT_NK):
        aps[f"xk{s}"] = nc.dram_tensor(f"xk{s}", [66, nk], F32R,
                                       kind="ExternalInput").ap()
        aps[f"xq{s}"] = nc.dram_tensor(f"xq{s}", [64, NQ], F32R,
                                       kind="ExternalInput").ap()
        aps[f"xkt{s}"] = nc.dram_tensor(f"xkt{s}", [128, (nk // 128) * 65],
                                        BF16, kind="ExternalInput").ap()
    aps["wpk"] = nc.dram_tensor("wpk", [128, 67], F32R,
                                kind="ExternalInput").ap()
    aps["auxb"] = nc.dram_tensor("auxb", [128, 192], BF16,
                                 kind="ExternalInput").ap()
    aps["o"] = nc.dram_tensor("o", [9, 64, NQ], F32, kind="ExternalOutput").ap()

    with tile.TileContext(nc) as tc:
        with ExitStack() as ctx:
            _emit(nc, tc, ctx, aps, reps)
    nc.compile()
    _CACHE[reps] = nc
    return nc


def _host_inputs(x, Wq, bq, Wk, bk, Wv, bv):
    x = np.asarray(x, np.float32)
    Wq = np.asarray(Wq, np.float32)
    Wk = np.asarray(Wk, np.float32)
    Wv = np.asarray(Wv, np.float32)
    bq = np.asarray(bq, np.float32)
    bk = np.asarray(bk, np.float32)
    bv = np.asarray(bv, np.float32)

    wpk = np.zeros((128, 67), np.float32)
    wpk[0:64, 0:64] = Wq.T @ Wk     # A_lhsT = (Wk^T Wq)^T
    wpk[0:64, 64] = Wq.T @ bk
    # query input is xq' = xq + bv (residual add applies bv for free);
    # correct the biases for the Wq/Wk projections accordingly
    bqv = bq - Wq @ bv
    wpk[0:64, 66] = Wk.T @ bqv
    wpk[64, 66] = bk @ bqv
    wpk[65, 66] = 1.0               # turns the mask channel on
    auxb = np.zeros((128, 192), np.float32)
    auxb[:, 0:128] = np.eye(128, dtype=np.float32)
    auxb[0:64, 128:192] = Wv.T
    auxb = auxb.astype(ml_dtypes.bfloat16)
    shared = {"wpk": wpk, "auxb": auxb}
    in_maps = []
    for c in range(N_CORES):
        m = dict(shared)
        for s, (b, i, j, z) in enumerate(TASKS[c]):
            nk_slot = SLOT_NK[s]
            sx, dx = _win(i)
            sy, dy = _win(j)
            sz, dz = _win_z(z)
            win = x[b, :, sx:sx + dx, sy:sy + dy, sz:sz + dz]
            nk = dx * dy * dz
            xkb = np.zeros((66, nk_slot), np.float32)
            xkb[0:64, :nk] = win.reshape(64, nk)
            xkb[64, :nk] = 1.0
            xkb[65, nk:] = MASK_NEG
            m[f"xk{s}"] = xkb
            mt = nk_slot // 128
            # xkt: (128 keys, m-tile, 64 ch + ones col), bf16
            xkt = np.zeros((128, mt, 65), np.float32)
            xkt[:, :, 0:64] = xkb[0:64].reshape(64, mt, 128).transpose(2, 1, 0)
            xkt[:, :, 64] = 1.0
            # padded keys: zero data so their (vanishing) weights stay clean
            if nk < nk_slot:
                flat = xkt.transpose(1, 0, 2).reshape(mt * 128, 65)
                flat[nk:, 0:64] = 0.0
                xkt = flat.reshape(mt, 128, 65).transpose(1, 0, 2)
            m[f"xkt{s}"] = np.ascontiguousarray(
                xkt.reshape(128, mt * 65)).astype(ml_dtypes.bfloat16)
            m[f"xq{s}"] = np.ascontiguousarray(
                win[:, 0:16, 0:16, 0:4].reshape(64, NQ)) + bv[:, None]
        in_maps.append(m)
    return in_maps


def _scatter(results):
    out = np.empty((B, C, H, W, T), np.float32)
    for c in range(N_CORES):
        o = results[c]["o"]
        for s, (b, i, j, z) in enumerate(TASKS[c]):
            sx, _ = _win(i)
            sy, _ = _win(j)
            sz, _ = _win_z(z)
            blk = o[s].reshape(64, 16, 16, 4)
            out[b, :, sx:sx + 16, sy:sy + 16, sz:sz + 4] = blk
    return out


def _ensure_axon():
    # The axon PJRT plugin is registered by sitecustomize at interpreter
    # start; if a caller pinned JAX_PLATFORMS=cpu before jax init, try to
    # re-enable the axon backend (run_bass_via_pjrt needs 8 trn2 devices).
    import jax

    try:
        if any(d.platform == "axon" for d in jax.devices()):
            return
    except Exception:
        pass
    try:
        jax.config.update("jax_platforms", "axon,cpu")
        jax.extend.backend.clear_backends()
    except Exception:
        pass


def run(x, Wq, bq, Wk, bk, Wv, bv, reps=1):
    _ensure_axon()
    nc = _build(reps)
    in_maps = _host_inputs(x, Wq, bq, Wk, bk, Wv, bv)
    res = run_bass_kernel_spmd(nc, in_maps, core_ids=list(range(N_CORES)))
    return _scatter(res.results), res


def kernel(x, Wq, bq, Wk, bk, Wv, bv):
    out, _ = run(x, Wq, bq, Wk, bk, Wv, bv,
                 reps=int(os.environ.get("KREP", "1")))
    return out
